# revision 18
# baseline (speedup 1.0000x reference)
"""Trainium2 Bass kernel for the Ablock spatial paradigm.

Reference computation (per sample, C=320 channels of 128x128):
    f    = silu(lem(x))
    fatt = lem(sigmoid(f) - 0.5)
    out  = (f + x) * fatt
where lem applies a per-channel circular 1-pixel shift S_c chosen by c%5:
    0: roll -1 along W   1: roll +1 along W
    2: roll -1 along H   3: roll +1 along H   4: identity

Because S_c commutes with elementwise ops and sigmoid(z)-0.5 = 0.5*tanh(z/2):
    u = silu(x);  w = tanh(u/2)
    out = (S u + x) * (0.5 * S^2 w)
so the only data movement is the shifts; silu and tanh share one ACT
table set (silu_and_others) so there are no activation-table reloads.

Sharding: pure data-parallel, one batch sample per NeuronCore (B=8).

Per-core layout: channels are processed in groups of G=16 with the same
shift type, tile [128, 2048]. Channel k of a tile occupies the 32
partitions [32*(k%4), 32*(k%4)+32) and the 512-float free block
[(k//4)*512, ...): partition p holds image rows 4b..4b+3 (b = p%32) as
four contiguous 128-float rows. Each channel is a 32x2KB DMA (2KB
contiguous runs in both DRAM and SBUF - full DMA descriptor
efficiency, vs 512B rows if H were mapped directly to partitions).

Shift handling in this layout:
  - W shifts: pure free-dim access-pattern offsets (main + wrap ops).
  - H shifts: rows move by r -> r+d within a partition (free offset
    +-128*d) except rows crossing the 4-row block boundary, which move
    to the neighboring partition. That cross-partition quarter (S) or
    half (S^2) is produced by a single TensorE matmul against a
    block-diagonal 32x32-circulant permutation matrix (circular wrap
    included), landing in PSUM and consumed directly by the DVE op.
"""

import numpy as np

import concourse.bacc as bacc
import concourse.mybir as mybir
from concourse.bass_utils import run_bass_kernel_spmd
from concourse.tile import TileContext

B, C, H, W = 8, 320, 128, 128
G = 16  # channels of one shift type per tile
F = G * W  # tile free size (2048)
NTYPE_CH = C // 5  # channels per shift type (64)
NCHUNK = NTYPE_CH // G  # tiles per type (4)
KP = 4  # partition groups (channels stacked along partitions)
KF = G // KP  # free blocks per tile
PB = H // KP  # partition block height (32) = image rows/4
FP32 = mybir.dt.float32
AOP = mybir.AluOpType


def _emit(nc, tc, x_d, o_d, p_d, f_func=None):
    act = mybir.ActivationFunctionType
    if f_func is None:
        f_func = act.Silu
    with (
        tc.tile_pool(name="pp", bufs=1) as pp,
        tc.tile_pool(name="xp", bufs=5) as xp,
        tc.tile_pool(name="up", bufs=3) as up,
        tc.tile_pool(name="wp", bufs=2) as wp,
        tc.tile_pool(name="ap", bufs=2) as ap_,
        tc.tile_pool(name="op", bufs=5) as op_,
        tc.tile_pool(name="pup", bufs=2, space="PSUM") as pup,
        tc.tile_pool(name="pwp", bufs=4, space="PSUM") as pwp,
    ):
        # Block-diagonal permutation matrices (4 identical 32x32 circulant
        # blocks): as matmul stationary lhsT, (P_d.T @ u)[p,f] =
        # u[group(p)*32 + (p%32 + d) % 32, f] for d = +1 / -1.
        pm = pp.tile([H, 2 * H], FP32, name="pm")
        nc.sync.dma_start(
            out=pm.rearrange("p (d i) -> p d i", d=2),
            in_=p_d.rearrange("d k i -> k d i"),
        )
        pm3 = pm.rearrange("p (d i) -> p d i", d=2)

        def views(t):
            # [p, kf, r, w] and [p, kf*r, w]
            return (
                t.rearrange("p (kf r w) -> p kf r w", kf=KF, w=W),
                t.rearrange("p (m w) -> p m w", w=W),
            )

        for g in range(NCHUNK):
            for r in range(5):
                c0 = r + 5 * G * g
                xt = xp.tile([H, F], FP32, name="xt")
                # One DMA per 32-partition group: 4 channels (stride 4 types
                # apart within the group) -> [32 partitions, 4 x 512] with
                # 2KB contiguous runs on both sides.
                for kp in range(KP):
                    cs = c0 + 5 * kp
                    srcg = (
                        x_d[cs : cs + 20 * (KF - 1) + 1 : 20]
                        .rearrange("k h w -> k (h w)")
                        .rearrange("k (b f) -> b k f", f=4 * W)
                    )
                    nc.sync.dma_start(
                        out=xt[kp * PB : (kp + 1) * PB, :].rearrange(
                            "b (kf f) -> b kf f", f=4 * W
                        ),
                        in_=srcg,
                    )

                u = up.tile([H, F], FP32, name="u")
                nc.scalar.activation(u, xt, f_func)
                w = wp.tile([H, F], FP32, name="w")
                nc.scalar.activation(w, u, act.Tanh, scale=0.5)

                a = ap_.tile([H, F], FP32, name="a")
                o = op_.tile([H, F], FP32, name="o")
                x4, x3 = views(xt)
                u4, u3 = views(u)
                w4, w3 = views(w)
                a4, a3 = views(a)
                o4, o3 = views(o)

                if r == 0:
                    # S: out(.,j) = in(.,j+1) along W
                    nc.vector.tensor_tensor(
                        a3[:, :, 0:127], u3[:, :, 1:128], x3[:, :, 0:127], AOP.add
                    )
                    nc.vector.tensor_tensor(
                        a3[:, :, 127:128], u3[:, :, 0:1], x3[:, :, 127:128], AOP.add
                    )
                    nc.vector.scalar_tensor_tensor(
                        o3[:, :, 0:126], w3[:, :, 2:128], 0.5, a3[:, :, 0:126],
                        AOP.mult, AOP.mult,
                    )
                    nc.vector.scalar_tensor_tensor(
                        o3[:, :, 126:128], w3[:, :, 0:2], 0.5, a3[:, :, 126:128],
                        AOP.mult, AOP.mult,
                    )
                elif r == 1:
                    # S: out(.,j) = in(.,j-1) along W
                    nc.vector.tensor_tensor(
                        a3[:, :, 1:128], u3[:, :, 0:127], x3[:, :, 1:128], AOP.add
                    )
                    nc.vector.tensor_tensor(
                        a3[:, :, 0:1], u3[:, :, 127:128], x3[:, :, 0:1], AOP.add
                    )
                    nc.vector.scalar_tensor_tensor(
                        o3[:, :, 2:128], w3[:, :, 0:126], 0.5, a3[:, :, 2:128],
                        AOP.mult, AOP.mult,
                    )
                    nc.vector.scalar_tensor_tensor(
                        o3[:, :, 0:2], w3[:, :, 126:128], 0.5, a3[:, :, 0:2],
                        AOP.mult, AOP.mult,
                    )
                elif r == 2:
                    # S: out(h) = in(h+1) along H. Rows r<3: same partition,
                    # free +128. Row r=3: next partition's r=0 via matmul.
                    nc.vector.tensor_tensor(
                        a4[:, :, 0:3, :], u4[:, :, 1:4, :], x4[:, :, 0:3, :], AOP.add
                    )
                    pu = pup.tile([H, 4 * W], FP32, name="pu")
                    nc.tensor.matmul(
                        pu, pm3[:, 0, :], u4[:, :, 0:1, :], start=True, stop=True
                    )
                    nc.vector.tensor_tensor(
                        a4[:, :, 3:4, :],
                        pu.rearrange("p (kf q w) -> p kf q w", kf=KF, w=W),
                        x4[:, :, 3:4, :],
                        AOP.add,
                    )
                    # S^2: rows r<2: free +256; rows r in {2,3}: next
                    # partition's rows {0,1}.
                    nc.vector.scalar_tensor_tensor(
                        o4[:, :, 0:2, :], w4[:, :, 2:4, :], 0.5, a4[:, :, 0:2, :],
                        AOP.mult, AOP.mult,
                    )
                    for j in range(2):
                        pw = pwp.tile([H, 4 * W], FP32, name="pw")
                        nc.tensor.matmul(
                            pw, pm3[:, 0, :], w4[:, :, j : j + 1, :],
                            start=True, stop=True,
                        )
                        nc.vector.scalar_tensor_tensor(
                            o4[:, :, 2 + j : 3 + j, :],
                            pw.rearrange("p (kf q w) -> p kf q w", kf=KF, w=W),
                            0.5,
                            a4[:, :, 2 + j : 3 + j, :],
                            AOP.mult, AOP.mult,
                        )
                elif r == 3:
                    # S: out(h) = in(h-1). Rows r>0: free -128. Row r=0:
                    # previous partition's r=3 via matmul.
                    nc.vector.tensor_tensor(
                        a4[:, :, 1:4, :], u4[:, :, 0:3, :], x4[:, :, 1:4, :], AOP.add
                    )
                    pu = pup.tile([H, 4 * W], FP32, name="pu")
                    nc.tensor.matmul(
                        pu, pm3[:, 1, :], u4[:, :, 3:4, :], start=True, stop=True
                    )
                    nc.vector.tensor_tensor(
                        a4[:, :, 0:1, :],
                        pu.rearrange("p (kf q w) -> p kf q w", kf=KF, w=W),
                        x4[:, :, 0:1, :],
                        AOP.add,
                    )
                    # S^2: rows r in {2,3}: free -256; rows {0,1}: previous
                    # partition's rows {2,3}.
                    nc.vector.scalar_tensor_tensor(
                        o4[:, :, 2:4, :], w4[:, :, 0:2, :], 0.5, a4[:, :, 2:4, :],
                        AOP.mult, AOP.mult,
                    )
                    for j in range(2):
                        pw = pwp.tile([H, 4 * W], FP32, name="pw")
                        nc.tensor.matmul(
                            pw, pm3[:, 1, :], w4[:, :, 2 + j : 3 + j, :],
                            start=True, stop=True,
                        )
                        nc.vector.scalar_tensor_tensor(
                            o4[:, :, j : j + 1, :],
                            pw.rearrange("p (kf q w) -> p kf q w", kf=KF, w=W),
                            0.5,
                            a4[:, :, j : j + 1, :],
                            AOP.mult, AOP.mult,
                        )
                else:
                    nc.vector.tensor_tensor(a, u, xt, AOP.add)
                    nc.vector.scalar_tensor_tensor(o, w, 0.5, a, AOP.mult, AOP.mult)

                # Stores on the ACT HWDGE ring so loads (sync ring) and
                # stores generate descriptors in parallel.
                for kp in range(KP):
                    cs = c0 + 5 * kp
                    dstg = (
                        o_d[cs : cs + 20 * (KF - 1) + 1 : 20]
                        .rearrange("k h w -> k (h w)")
                        .rearrange("k (b f) -> b k f", f=4 * W)
                    )
                    nc.scalar.dma_start(
                        out=dstg,
                        in_=o[kp * PB : (kp + 1) * PB, :].rearrange(
                            "b (kf f) -> b kf f", f=4 * W
                        ),
                    )


_NC_CACHE = {}


def _build(f_func=None):
    key = ("nc", str(f_func))
    if key in _NC_CACHE:
        return _NC_CACHE[key]
    nc = bacc.Bacc(
        "TRN2",
        target_bir_lowering=False,
        debug=False,
        enable_asserts=True,
        num_devices=B,
    )
    x_d = nc.dram_tensor("x", [C, H, W], FP32, kind="ExternalInput").ap()
    p_d = nc.dram_tensor("perm", [2, H, H], FP32, kind="ExternalInput").ap()
    o_d = nc.dram_tensor("out", [C, H, W], FP32, kind="ExternalOutput").ap()
    with TileContext(nc) as tc:
        _emit(nc, tc, x_d, o_d, p_d, f_func=f_func)
    nc.compile()
    _NC_CACHE[key] = nc
    return nc


def _perm_mats():
    pm = np.zeros((2, H, H), dtype=np.float32)
    i = np.arange(H)
    for d, delta in enumerate((1, -1)):
        pm[d, (i // PB) * PB + (i % PB + delta) % PB, i] = 1.0
    return pm


def run(x, trace=False, tmpdir=None):
    x = np.ascontiguousarray(np.asarray(x), dtype=np.float32)
    assert x.shape == (B, C, H, W), x.shape
    nc = _build()
    pm = _perm_mats()
    in_maps = [{"x": np.ascontiguousarray(x[i]), "perm": pm} for i in range(B)]
    res = run_bass_kernel_spmd(
        nc, in_maps, core_ids=list(range(B)), trace=trace, tmpdir=tmpdir
    )
    out = np.stack([res.results[i]["out"] for i in range(B)], axis=0)
    return out, res


def kernel(x):
    out, _ = run(x)
    return out


# revision 21
# speedup vs baseline: 1.4462x; 1.4462x over previous
"""Trainium2 Bass kernel for the Ablock spatial paradigm.

Reference computation (per sample, C=320 channels of 128x128):
    f    = silu(lem(x))
    fatt = lem(sigmoid(f) - 0.5)
    out  = (f + x) * fatt
where lem applies a per-channel circular 1-pixel shift S_c chosen by c%5:
    0: roll -1 along W   1: roll +1 along W
    2: roll -1 along H   3: roll +1 along H   4: identity

Because S_c commutes with elementwise ops and sigmoid(z)-0.5 = 0.5*tanh(z/2):
    u = silu(x);  w = tanh(u/2)
    out = (S u + x) * (0.5 * S^2 w)
so the only data movement is the shifts; silu and tanh share one ACT
table set (silu_and_others) so there are no activation-table reloads.

Sharding: pure data-parallel, one batch sample per NeuronCore (B=8).

Per-core layout: channels are processed in groups of G=16 with the same
shift type, tile [128, 2048]. Channel k of a tile occupies partitions
[8k, 8k+8); partition p holds image rows 16*(p%8) .. 16*(p%8)+15 as 16
contiguous 128-float rows (8KB). Each tile load/store is then a single
3-dim DMA with 8KB contiguous runs on both the DRAM and SBUF side.

Shift handling in this layout:
  - W shifts: pure free-dim access-pattern offsets (main + wrap ops).
  - H shifts: rows move by r -> r+d within a partition (free offset
    +-128*d) except the d rows crossing the 16-row partition boundary,
    which move to the neighboring partition. That cross-partition
    1/16th (S) or 2/16ths (S^2) of the volume is produced by one small
    TensorE matmul against a block-diagonal 8x8-circulant permutation
    matrix (circular wrap included), landing in PSUM and consumed
    directly by the DVE op.
"""

import numpy as np

import concourse.bacc as bacc
import concourse.mybir as mybir
from concourse.bass_utils import run_bass_kernel_spmd
from concourse.tile import TileContext

B, C, H, W = 8, 320, 128, 128
G = 16  # channels of one shift type per tile
F = G * W  # tile free size (2048)
NTYPE_CH = C // 5  # channels per shift type (64)
NCHUNK = NTYPE_CH // G  # tiles per type (4)
PPC = H // G  # partitions per channel (8)
RPP = H // PPC  # image rows per partition (16)
FP32 = mybir.dt.float32
AOP = mybir.AluOpType


def _emit(nc, tc, x_d, o_d, p_d, f_func=None):
    act = mybir.ActivationFunctionType
    if f_func is None:
        f_func = act.Silu
    with (
        tc.tile_pool(name="pp", bufs=1) as pp,
        tc.tile_pool(name="xp", bufs=5) as xp,
        tc.tile_pool(name="up", bufs=3) as up,
        tc.tile_pool(name="wp", bufs=2) as wp,
        tc.tile_pool(name="ap", bufs=2) as ap_,
        tc.tile_pool(name="op", bufs=5) as op_,
        tc.tile_pool(name="pup", bufs=2, space="PSUM") as pup,
        tc.tile_pool(name="pwp", bufs=2, space="PSUM") as pwp,
    ):
        # Block-diagonal permutation matrices (16 identical 8x8 circulant
        # blocks): as matmul stationary lhsT, (P_d.T @ u)[p,f] =
        # u[group(p)*8 + (p%8 + d) % 8, f] for d = +1 / -1.
        pm = pp.tile([H, 2 * H], FP32, name="pm")
        nc.sync.dma_start(
            out=pm.rearrange("p (d i) -> p d i", d=2),
            in_=p_d.rearrange("d k i -> k d i"),
        )
        pm3 = pm.rearrange("p (d i) -> p d i", d=2)

        for g in range(NCHUNK):
            for r in range(5):
                c0 = r + 5 * G * g
                src = (
                    x_d[c0 : c0 + 5 * (G - 1) + 1 : 5]
                    .rearrange("k h w -> k (h w)")
                    .rearrange("k (q f) -> k q f", f=RPP * W)
                )
                xt = xp.tile([H, F], FP32, name="xt")
                nc.sync.dma_start(out=xt, in_=src)

                u = up.tile([H, F], FP32, name="u")
                nc.scalar.activation(u, xt, f_func)
                w = wp.tile([H, F], FP32, name="w")
                nc.scalar.activation(w, u, act.Tanh, scale=0.5)

                a = ap_.tile([H, F], FP32, name="a")
                o = op_.tile([H, F], FP32, name="o")
                # [partition, image-row-in-partition, w]
                x3 = xt.rearrange("p (m w) -> p m w", w=W)
                u3 = u.rearrange("p (m w) -> p m w", w=W)
                w3 = w.rearrange("p (m w) -> p m w", w=W)
                a3 = a.rearrange("p (m w) -> p m w", w=W)
                o3 = o.rearrange("p (m w) -> p m w", w=W)

                if r == 0:
                    # S: out(.,j) = in(.,j+1) along W
                    nc.vector.tensor_tensor(
                        a3[:, :, 0:127], u3[:, :, 1:128], x3[:, :, 0:127], AOP.add
                    )
                    nc.vector.tensor_tensor(
                        a3[:, :, 127:128], u3[:, :, 0:1], x3[:, :, 127:128], AOP.add
                    )
                    nc.vector.scalar_tensor_tensor(
                        o3[:, :, 0:126], w3[:, :, 2:128], 0.5, a3[:, :, 0:126],
                        AOP.mult, AOP.mult,
                    )
                    nc.vector.scalar_tensor_tensor(
                        o3[:, :, 126:128], w3[:, :, 0:2], 0.5, a3[:, :, 126:128],
                        AOP.mult, AOP.mult,
                    )
                elif r == 1:
                    # S: out(.,j) = in(.,j-1) along W
                    nc.vector.tensor_tensor(
                        a3[:, :, 1:128], u3[:, :, 0:127], x3[:, :, 1:128], AOP.add
                    )
                    nc.vector.tensor_tensor(
                        a3[:, :, 0:1], u3[:, :, 127:128], x3[:, :, 0:1], AOP.add
                    )
                    nc.vector.scalar_tensor_tensor(
                        o3[:, :, 2:128], w3[:, :, 0:126], 0.5, a3[:, :, 2:128],
                        AOP.mult, AOP.mult,
                    )
                    nc.vector.scalar_tensor_tensor(
                        o3[:, :, 0:2], w3[:, :, 126:128], 0.5, a3[:, :, 0:2],
                        AOP.mult, AOP.mult,
                    )
                elif r == 2:
                    # S: out(h) = in(h+1). Rows 0..14: same partition, free
                    # +128. Row 15: next partition's row 0 via matmul.
                    nc.vector.tensor_tensor(
                        a3[:, 0:15, :], u3[:, 1:16, :], x3[:, 0:15, :], AOP.add
                    )
                    pu = pup.tile([H, W], FP32, name="pu")
                    nc.tensor.matmul(
                        pu, pm3[:, 0, :], u3[:, 0:1, :], start=True, stop=True
                    )
                    nc.vector.tensor_tensor(
                        a3[:, 15:16, :],
                        pu.rearrange("p (q w) -> p q w", q=1),
                        x3[:, 15:16, :],
                        AOP.add,
                    )
                    # S^2: rows 0..13: free +256; rows 14,15: next
                    # partition's rows 0,1.
                    nc.vector.scalar_tensor_tensor(
                        o3[:, 0:14, :], w3[:, 2:16, :], 0.5, a3[:, 0:14, :],
                        AOP.mult, AOP.mult,
                    )
                    pw = pwp.tile([H, 2 * W], FP32, name="pw")
                    nc.tensor.matmul(
                        pw, pm3[:, 0, :], w3[:, 0:2, :], start=True, stop=True
                    )
                    nc.vector.scalar_tensor_tensor(
                        o3[:, 14:16, :],
                        pw.rearrange("p (q w) -> p q w", q=2),
                        0.5,
                        a3[:, 14:16, :],
                        AOP.mult, AOP.mult,
                    )
                elif r == 3:
                    # S: out(h) = in(h-1). Rows 1..15: free -128. Row 0:
                    # previous partition's row 15 via matmul.
                    nc.vector.tensor_tensor(
                        a3[:, 1:16, :], u3[:, 0:15, :], x3[:, 1:16, :], AOP.add
                    )
                    pu = pup.tile([H, W], FP32, name="pu")
                    nc.tensor.matmul(
                        pu, pm3[:, 1, :], u3[:, 15:16, :], start=True, stop=True
                    )
                    nc.vector.tensor_tensor(
                        a3[:, 0:1, :],
                        pu.rearrange("p (q w) -> p q w", q=1),
                        x3[:, 0:1, :],
                        AOP.add,
                    )
                    # S^2: rows 2..15: free -256; rows 0,1: previous
                    # partition's rows 14,15.
                    nc.vector.scalar_tensor_tensor(
                        o3[:, 2:16, :], w3[:, 0:14, :], 0.5, a3[:, 2:16, :],
                        AOP.mult, AOP.mult,
                    )
                    pw = pwp.tile([H, 2 * W], FP32, name="pw")
                    nc.tensor.matmul(
                        pw, pm3[:, 1, :], w3[:, 14:16, :], start=True, stop=True
                    )
                    nc.vector.scalar_tensor_tensor(
                        o3[:, 0:2, :],
                        pw.rearrange("p (q w) -> p q w", q=2),
                        0.5,
                        a3[:, 0:2, :],
                        AOP.mult, AOP.mult,
                    )
                else:
                    nc.vector.tensor_tensor(a, u, xt, AOP.add)
                    nc.vector.scalar_tensor_tensor(o, w, 0.5, a, AOP.mult, AOP.mult)

                dst = (
                    o_d[c0 : c0 + 5 * (G - 1) + 1 : 5]
                    .rearrange("k h w -> k (h w)")
                    .rearrange("k (q f) -> k q f", f=RPP * W)
                )
                # Stores on the ACT HWDGE ring so loads (sync ring) and
                # stores generate descriptors in parallel.
                nc.scalar.dma_start(out=dst, in_=o)


_NC_CACHE = {}


def _build(f_func=None):
    key = ("nc", str(f_func))
    if key in _NC_CACHE:
        return _NC_CACHE[key]
    nc = bacc.Bacc(
        "TRN2",
        target_bir_lowering=False,
        debug=False,
        enable_asserts=True,
        num_devices=B,
    )
    x_d = nc.dram_tensor("x", [C, H, W], FP32, kind="ExternalInput").ap()
    p_d = nc.dram_tensor("perm", [2, H, H], FP32, kind="ExternalInput").ap()
    o_d = nc.dram_tensor("out", [C, H, W], FP32, kind="ExternalOutput").ap()
    with TileContext(nc) as tc:
        _emit(nc, tc, x_d, o_d, p_d, f_func=f_func)
    nc.compile()
    _NC_CACHE[key] = nc
    return nc


def _perm_mats():
    pm = np.zeros((2, H, H), dtype=np.float32)
    i = np.arange(H)
    for d, delta in enumerate((1, -1)):
        pm[d, (i // PPC) * PPC + (i % PPC + delta) % PPC, i] = 1.0
    return pm


def run(x, trace=False, tmpdir=None):
    x = np.ascontiguousarray(np.asarray(x), dtype=np.float32)
    assert x.shape == (B, C, H, W), x.shape
    nc = _build()
    pm = _perm_mats()
    in_maps = [{"x": np.ascontiguousarray(x[i]), "perm": pm} for i in range(B)]
    res = run_bass_kernel_spmd(
        nc, in_maps, core_ids=list(range(B)), trace=trace, tmpdir=tmpdir
    )
    out = np.stack([res.results[i]["out"] for i in range(B)], axis=0)
    return out, res


def kernel(x):
    out, _ = run(x)
    return out


# revision 23
# speedup vs baseline: 1.4967x; 1.0349x over previous
"""Trainium2 Bass kernel for the Ablock spatial paradigm.

Reference computation (per sample, C=320 channels of 128x128):
    f    = silu(lem(x))
    fatt = lem(sigmoid(f) - 0.5)
    out  = (f + x) * fatt
where lem applies a per-channel circular 1-pixel shift S_c chosen by c%5:
    0: roll -1 along W   1: roll +1 along W
    2: roll -1 along H   3: roll +1 along H   4: identity

Because S_c commutes with elementwise ops and sigmoid(z)-0.5 = 0.5*tanh(z/2):
    u = silu(x);  w = tanh(u/2)
    out = (S u + x) * (0.5 * S^2 w)
so the only data movement is the shifts; silu and tanh share one ACT
table set (silu_and_others) so there are no activation-table reloads.

Sharding: pure data-parallel, one batch sample per NeuronCore (B=8).

Per-core layout: channels are processed in groups of G=16 with the same
shift type, tile [128, 2048]. Channel k of a tile occupies partitions
[8k, 8k+8); partition p holds image rows 16*(p%8) .. 16*(p%8)+15 as 16
contiguous 128-float rows (8KB). Each tile load/store is then a single
3-dim DMA with 8KB contiguous runs on both the DRAM and SBUF side.

Shift handling in this layout:
  - W shifts: pure free-dim access-pattern offsets (main + wrap ops).
  - H shifts: rows move by r -> r+d within a partition (free offset
    +-128*d) except the d rows crossing the 16-row partition boundary,
    which move to the neighboring partition. That cross-partition
    1/16th (S) or 2/16ths (S^2) of the volume is produced by one small
    TensorE matmul against a block-diagonal 8x8-circulant permutation
    matrix (circular wrap included), landing in PSUM and consumed
    directly by the DVE op.
"""

import numpy as np

import concourse.bacc as bacc
import concourse.mybir as mybir
from concourse.bass_utils import run_bass_kernel_spmd
from concourse.tile import TileContext

B, C, H, W = 8, 320, 128, 128
G = 16  # channels of one shift type per tile
F = G * W  # tile free size (2048)
NTYPE_CH = C // 5  # channels per shift type (64)
NCHUNK = NTYPE_CH // G  # tiles per type (4)
PPC = H // G  # partitions per channel (8)
RPP = H // PPC  # image rows per partition (16)
FP32 = mybir.dt.float32
AOP = mybir.AluOpType


def _emit(nc, tc, x_d, o_d, p_d, f_func=None):
    act = mybir.ActivationFunctionType
    if f_func is None:
        f_func = act.Silu
    with (
        tc.tile_pool(name="pp", bufs=1) as pp,
        tc.tile_pool(name="xp", bufs=5) as xp,
        tc.tile_pool(name="up", bufs=3) as up,
        tc.tile_pool(name="wp", bufs=2) as wp,
        tc.tile_pool(name="ap", bufs=2) as ap_,
        tc.tile_pool(name="op", bufs=5) as op_,
        tc.tile_pool(name="pup", bufs=2, space="PSUM") as pup,
        tc.tile_pool(name="pwp", bufs=2, space="PSUM") as pwp,
    ):
        # Block-diagonal permutation matrices (16 identical 8x8 circulant
        # blocks): as matmul stationary lhsT, (P_d.T @ u)[p,f] =
        # u[group(p)*8 + (p%8 + d) % 8, f] for d = +1 / -1.
        pm = pp.tile([H, 2 * H], FP32, name="pm")
        nc.sync.dma_start(
            out=pm.rearrange("p (d i) -> p d i", d=2),
            in_=p_d.rearrange("d k i -> k d i"),
        )
        pm3 = pm.rearrange("p (d i) -> p d i", d=2)

        for g in range(NCHUNK):
            for r in range(5):
                c0 = r + 5 * G * g
                src = (
                    x_d[c0 : c0 + 5 * (G - 1) + 1 : 5]
                    .rearrange("k h w -> k (h w)")
                    .rearrange("k (q f) -> k q f", f=RPP * W)
                )
                xt = xp.tile([H, F], FP32, name="xt")
                nc.sync.dma_start(out=xt, in_=src, single_packet=True)

                u = up.tile([H, F], FP32, name="u")
                nc.scalar.activation(u, xt, f_func)
                w = wp.tile([H, F], FP32, name="w")
                nc.scalar.activation(w, u, act.Tanh, scale=0.5)

                a = ap_.tile([H, F], FP32, name="a")
                o = op_.tile([H, F], FP32, name="o")
                # [partition, image-row-in-partition, w]
                x3 = xt.rearrange("p (m w) -> p m w", w=W)
                u3 = u.rearrange("p (m w) -> p m w", w=W)
                w3 = w.rearrange("p (m w) -> p m w", w=W)
                a3 = a.rearrange("p (m w) -> p m w", w=W)
                o3 = o.rearrange("p (m w) -> p m w", w=W)

                if r == 0:
                    # S: out(.,j) = in(.,j+1) along W
                    nc.vector.tensor_tensor(
                        a3[:, :, 0:127], u3[:, :, 1:128], x3[:, :, 0:127], AOP.add
                    )
                    nc.vector.tensor_tensor(
                        a3[:, :, 127:128], u3[:, :, 0:1], x3[:, :, 127:128], AOP.add
                    )
                    nc.vector.scalar_tensor_tensor(
                        o3[:, :, 0:126], w3[:, :, 2:128], 0.5, a3[:, :, 0:126],
                        AOP.mult, AOP.mult,
                    )
                    nc.vector.scalar_tensor_tensor(
                        o3[:, :, 126:128], w3[:, :, 0:2], 0.5, a3[:, :, 126:128],
                        AOP.mult, AOP.mult,
                    )
                elif r == 1:
                    # S: out(.,j) = in(.,j-1) along W
                    nc.vector.tensor_tensor(
                        a3[:, :, 1:128], u3[:, :, 0:127], x3[:, :, 1:128], AOP.add
                    )
                    nc.vector.tensor_tensor(
                        a3[:, :, 0:1], u3[:, :, 127:128], x3[:, :, 0:1], AOP.add
                    )
                    nc.vector.scalar_tensor_tensor(
                        o3[:, :, 2:128], w3[:, :, 0:126], 0.5, a3[:, :, 2:128],
                        AOP.mult, AOP.mult,
                    )
                    nc.vector.scalar_tensor_tensor(
                        o3[:, :, 0:2], w3[:, :, 126:128], 0.5, a3[:, :, 0:2],
                        AOP.mult, AOP.mult,
                    )
                elif r == 2:
                    # S: out(h) = in(h+1). Rows 0..14: same partition, free
                    # +128. Row 15: next partition's row 0 via matmul.
                    nc.vector.tensor_tensor(
                        a3[:, 0:15, :], u3[:, 1:16, :], x3[:, 0:15, :], AOP.add
                    )
                    pu = pup.tile([H, W], FP32, name="pu")
                    nc.tensor.matmul(
                        pu, pm3[:, 0, :], u3[:, 0:1, :], start=True, stop=True
                    )
                    nc.vector.tensor_tensor(
                        a3[:, 15:16, :],
                        pu.rearrange("p (q w) -> p q w", q=1),
                        x3[:, 15:16, :],
                        AOP.add,
                    )
                    # S^2: rows 0..13: free +256; rows 14,15: next
                    # partition's rows 0,1.
                    nc.vector.scalar_tensor_tensor(
                        o3[:, 0:14, :], w3[:, 2:16, :], 0.5, a3[:, 0:14, :],
                        AOP.mult, AOP.mult,
                    )
                    pw = pwp.tile([H, 2 * W], FP32, name="pw")
                    nc.tensor.matmul(
                        pw, pm3[:, 0, :], w3[:, 0:2, :], start=True, stop=True
                    )
                    nc.vector.scalar_tensor_tensor(
                        o3[:, 14:16, :],
                        pw.rearrange("p (q w) -> p q w", q=2),
                        0.5,
                        a3[:, 14:16, :],
                        AOP.mult, AOP.mult,
                    )
                elif r == 3:
                    # S: out(h) = in(h-1). Rows 1..15: free -128. Row 0:
                    # previous partition's row 15 via matmul.
                    nc.vector.tensor_tensor(
                        a3[:, 1:16, :], u3[:, 0:15, :], x3[:, 1:16, :], AOP.add
                    )
                    pu = pup.tile([H, W], FP32, name="pu")
                    nc.tensor.matmul(
                        pu, pm3[:, 1, :], u3[:, 15:16, :], start=True, stop=True
                    )
                    nc.vector.tensor_tensor(
                        a3[:, 0:1, :],
                        pu.rearrange("p (q w) -> p q w", q=1),
                        x3[:, 0:1, :],
                        AOP.add,
                    )
                    # S^2: rows 2..15: free -256; rows 0,1: previous
                    # partition's rows 14,15.
                    nc.vector.scalar_tensor_tensor(
                        o3[:, 2:16, :], w3[:, 0:14, :], 0.5, a3[:, 2:16, :],
                        AOP.mult, AOP.mult,
                    )
                    pw = pwp.tile([H, 2 * W], FP32, name="pw")
                    nc.tensor.matmul(
                        pw, pm3[:, 1, :], w3[:, 14:16, :], start=True, stop=True
                    )
                    nc.vector.scalar_tensor_tensor(
                        o3[:, 0:2, :],
                        pw.rearrange("p (q w) -> p q w", q=2),
                        0.5,
                        a3[:, 0:2, :],
                        AOP.mult, AOP.mult,
                    )
                else:
                    nc.vector.tensor_tensor(a, u, xt, AOP.add)
                    nc.vector.scalar_tensor_tensor(o, w, 0.5, a, AOP.mult, AOP.mult)

                dst = (
                    o_d[c0 : c0 + 5 * (G - 1) + 1 : 5]
                    .rearrange("k h w -> k (h w)")
                    .rearrange("k (q f) -> k q f", f=RPP * W)
                )
                # Stores on the ACT HWDGE ring so loads (sync ring) and
                # stores generate descriptors in parallel.
                nc.scalar.dma_start(out=dst, in_=o, single_packet=True)


_NC_CACHE = {}


def _build(f_func=None):
    key = ("nc", str(f_func))
    if key in _NC_CACHE:
        return _NC_CACHE[key]
    nc = bacc.Bacc(
        "TRN2",
        target_bir_lowering=False,
        debug=False,
        enable_asserts=True,
        num_devices=B,
    )
    x_d = nc.dram_tensor("x", [C, H, W], FP32, kind="ExternalInput").ap()
    p_d = nc.dram_tensor("perm", [2, H, H], FP32, kind="ExternalInput").ap()
    o_d = nc.dram_tensor("out", [C, H, W], FP32, kind="ExternalOutput").ap()
    with TileContext(nc) as tc:
        _emit(nc, tc, x_d, o_d, p_d, f_func=f_func)
    nc.compile()
    _NC_CACHE[key] = nc
    return nc


def _perm_mats():
    pm = np.zeros((2, H, H), dtype=np.float32)
    i = np.arange(H)
    for d, delta in enumerate((1, -1)):
        pm[d, (i // PPC) * PPC + (i % PPC + delta) % PPC, i] = 1.0
    return pm


def run(x, trace=False, tmpdir=None):
    x = np.ascontiguousarray(np.asarray(x), dtype=np.float32)
    assert x.shape == (B, C, H, W), x.shape
    nc = _build()
    pm = _perm_mats()
    in_maps = [{"x": np.ascontiguousarray(x[i]), "perm": pm} for i in range(B)]
    res = run_bass_kernel_spmd(
        nc, in_maps, core_ids=list(range(B)), trace=trace, tmpdir=tmpdir
    )
    out = np.stack([res.results[i]["out"] for i in range(B)], axis=0)
    return out, res


def kernel(x):
    out, _ = run(x)
    return out


# revision 24
# speedup vs baseline: 1.5305x; 1.0225x over previous
"""Trainium2 Bass kernel for the Ablock spatial paradigm.

Reference computation (per sample, C=320 channels of 128x128):
    f    = silu(lem(x))
    fatt = lem(sigmoid(f) - 0.5)
    out  = (f + x) * fatt
where lem applies a per-channel circular 1-pixel shift S_c chosen by c%5:
    0: roll -1 along W   1: roll +1 along W
    2: roll -1 along H   3: roll +1 along H   4: identity

Because S_c commutes with elementwise ops and sigmoid(z)-0.5 = 0.5*tanh(z/2):
    u = silu(x);  w = tanh(u/2)
    out = (S u + x) * (0.5 * S^2 w)
so the only data movement is the shifts; silu and tanh share one ACT
table set (silu_and_others) so there are no activation-table reloads.

Sharding: pure data-parallel, one batch sample per NeuronCore (B=8).

Per-core layout: channels are processed in groups of G=16 with the same
shift type, tile [128, 2048]. Channel k of a tile occupies partitions
[8k, 8k+8); partition p holds image rows 16*(p%8) .. 16*(p%8)+15 as 16
contiguous 128-float rows (8KB). Each tile load/store is then a single
3-dim DMA with 8KB contiguous runs on both the DRAM and SBUF side.

Shift handling in this layout:
  - W shifts: pure free-dim access-pattern offsets (main + wrap ops).
  - H shifts: rows move by r -> r+d within a partition (free offset
    +-128*d) except the d rows crossing the 16-row partition boundary,
    which move to the neighboring partition. That cross-partition
    1/16th (S) or 2/16ths (S^2) of the volume is produced by one small
    TensorE matmul against a block-diagonal 8x8-circulant permutation
    matrix (circular wrap included), landing in PSUM and consumed
    directly by the DVE op.
"""

import numpy as np

import concourse.bacc as bacc
import concourse.mybir as mybir
from concourse.bass_utils import run_bass_kernel_spmd
from concourse.tile import TileContext

B, C, H, W = 8, 320, 128, 128
G = 16  # channels of one shift type per tile
F = G * W  # tile free size (2048)
NTYPE_CH = C // 5  # channels per shift type (64)
NCHUNK = NTYPE_CH // G  # tiles per type (4)
PPC = H // G  # partitions per channel (8)
RPP = H // PPC  # image rows per partition (16)
FP32 = mybir.dt.float32
AOP = mybir.AluOpType


def _emit(nc, tc, x_d, o_d, p_d, f_func=None):
    act = mybir.ActivationFunctionType
    if f_func is None:
        f_func = act.Silu
    with (
        tc.tile_pool(name="pp", bufs=1) as pp,
        tc.tile_pool(name="xp", bufs=7) as xp,
        tc.tile_pool(name="up", bufs=3) as up,
        tc.tile_pool(name="wp", bufs=2) as wp,
        tc.tile_pool(name="ap", bufs=2) as ap_,
        tc.tile_pool(name="op", bufs=7) as op_,
        tc.tile_pool(name="pup", bufs=2, space="PSUM") as pup,
        tc.tile_pool(name="pwp", bufs=2, space="PSUM") as pwp,
    ):
        # Block-diagonal permutation matrices (16 identical 8x8 circulant
        # blocks): as matmul stationary lhsT, (P_d.T @ u)[p,f] =
        # u[group(p)*8 + (p%8 + d) % 8, f] for d = +1 / -1.
        pm = pp.tile([H, 2 * H], FP32, name="pm")
        nc.sync.dma_start(
            out=pm.rearrange("p (d i) -> p d i", d=2),
            in_=p_d.rearrange("d k i -> k d i"),
        )
        pm3 = pm.rearrange("p (d i) -> p d i", d=2)

        for g in range(NCHUNK):
            for r in range(5):
                c0 = r + 5 * G * g
                src = (
                    x_d[c0 : c0 + 5 * (G - 1) + 1 : 5]
                    .rearrange("k h w -> k (h w)")
                    .rearrange("k (q f) -> k q f", f=RPP * W)
                )
                xt = xp.tile([H, F], FP32, name="xt")
                nc.sync.dma_start(out=xt, in_=src, single_packet=True)

                u = up.tile([H, F], FP32, name="u")
                nc.scalar.activation(u, xt, f_func)
                w = wp.tile([H, F], FP32, name="w")
                nc.scalar.activation(w, u, act.Tanh, scale=0.5)

                a = ap_.tile([H, F], FP32, name="a")
                o = op_.tile([H, F], FP32, name="o")
                # [partition, image-row-in-partition, w]
                x3 = xt.rearrange("p (m w) -> p m w", w=W)
                u3 = u.rearrange("p (m w) -> p m w", w=W)
                w3 = w.rearrange("p (m w) -> p m w", w=W)
                a3 = a.rearrange("p (m w) -> p m w", w=W)
                o3 = o.rearrange("p (m w) -> p m w", w=W)

                if r == 0:
                    # S: out(.,j) = in(.,j+1) along W
                    nc.vector.tensor_tensor(
                        a3[:, :, 0:127], u3[:, :, 1:128], x3[:, :, 0:127], AOP.add
                    )
                    nc.vector.tensor_tensor(
                        a3[:, :, 127:128], u3[:, :, 0:1], x3[:, :, 127:128], AOP.add
                    )
                    nc.vector.scalar_tensor_tensor(
                        o3[:, :, 0:126], w3[:, :, 2:128], 0.5, a3[:, :, 0:126],
                        AOP.mult, AOP.mult,
                    )
                    nc.vector.scalar_tensor_tensor(
                        o3[:, :, 126:128], w3[:, :, 0:2], 0.5, a3[:, :, 126:128],
                        AOP.mult, AOP.mult,
                    )
                elif r == 1:
                    # S: out(.,j) = in(.,j-1) along W
                    nc.vector.tensor_tensor(
                        a3[:, :, 1:128], u3[:, :, 0:127], x3[:, :, 1:128], AOP.add
                    )
                    nc.vector.tensor_tensor(
                        a3[:, :, 0:1], u3[:, :, 127:128], x3[:, :, 0:1], AOP.add
                    )
                    nc.vector.scalar_tensor_tensor(
                        o3[:, :, 2:128], w3[:, :, 0:126], 0.5, a3[:, :, 2:128],
                        AOP.mult, AOP.mult,
                    )
                    nc.vector.scalar_tensor_tensor(
                        o3[:, :, 0:2], w3[:, :, 126:128], 0.5, a3[:, :, 0:2],
                        AOP.mult, AOP.mult,
                    )
                elif r == 2:
                    # S: out(h) = in(h+1). Rows 0..14: same partition, free
                    # +128. Row 15: next partition's row 0 via matmul.
                    nc.vector.tensor_tensor(
                        a3[:, 0:15, :], u3[:, 1:16, :], x3[:, 0:15, :], AOP.add
                    )
                    pu = pup.tile([H, W], FP32, name="pu")
                    nc.tensor.matmul(
                        pu, pm3[:, 0, :], u3[:, 0:1, :], start=True, stop=True
                    )
                    nc.vector.tensor_tensor(
                        a3[:, 15:16, :],
                        pu.rearrange("p (q w) -> p q w", q=1),
                        x3[:, 15:16, :],
                        AOP.add,
                    )
                    # S^2: rows 0..13: free +256; rows 14,15: next
                    # partition's rows 0,1.
                    nc.vector.scalar_tensor_tensor(
                        o3[:, 0:14, :], w3[:, 2:16, :], 0.5, a3[:, 0:14, :],
                        AOP.mult, AOP.mult,
                    )
                    pw = pwp.tile([H, 2 * W], FP32, name="pw")
                    nc.tensor.matmul(
                        pw, pm3[:, 0, :], w3[:, 0:2, :], start=True, stop=True
                    )
                    nc.vector.scalar_tensor_tensor(
                        o3[:, 14:16, :],
                        pw.rearrange("p (q w) -> p q w", q=2),
                        0.5,
                        a3[:, 14:16, :],
                        AOP.mult, AOP.mult,
                    )
                elif r == 3:
                    # S: out(h) = in(h-1). Rows 1..15: free -128. Row 0:
                    # previous partition's row 15 via matmul.
                    nc.vector.tensor_tensor(
                        a3[:, 1:16, :], u3[:, 0:15, :], x3[:, 1:16, :], AOP.add
                    )
                    pu = pup.tile([H, W], FP32, name="pu")
                    nc.tensor.matmul(
                        pu, pm3[:, 1, :], u3[:, 15:16, :], start=True, stop=True
                    )
                    nc.vector.tensor_tensor(
                        a3[:, 0:1, :],
                        pu.rearrange("p (q w) -> p q w", q=1),
                        x3[:, 0:1, :],
                        AOP.add,
                    )
                    # S^2: rows 2..15: free -256; rows 0,1: previous
                    # partition's rows 14,15.
                    nc.vector.scalar_tensor_tensor(
                        o3[:, 2:16, :], w3[:, 0:14, :], 0.5, a3[:, 2:16, :],
                        AOP.mult, AOP.mult,
                    )
                    pw = pwp.tile([H, 2 * W], FP32, name="pw")
                    nc.tensor.matmul(
                        pw, pm3[:, 1, :], w3[:, 14:16, :], start=True, stop=True
                    )
                    nc.vector.scalar_tensor_tensor(
                        o3[:, 0:2, :],
                        pw.rearrange("p (q w) -> p q w", q=2),
                        0.5,
                        a3[:, 0:2, :],
                        AOP.mult, AOP.mult,
                    )
                else:
                    nc.vector.tensor_tensor(a, u, xt, AOP.add)
                    nc.vector.scalar_tensor_tensor(o, w, 0.5, a, AOP.mult, AOP.mult)

                dst = (
                    o_d[c0 : c0 + 5 * (G - 1) + 1 : 5]
                    .rearrange("k h w -> k (h w)")
                    .rearrange("k (q f) -> k q f", f=RPP * W)
                )
                # Stores on the ACT HWDGE ring so loads (sync ring) and
                # stores generate descriptors in parallel.
                nc.scalar.dma_start(out=dst, in_=o, single_packet=True)


_NC_CACHE = {}


def _build(f_func=None):
    key = ("nc", str(f_func))
    if key in _NC_CACHE:
        return _NC_CACHE[key]
    nc = bacc.Bacc(
        "TRN2",
        target_bir_lowering=False,
        debug=False,
        enable_asserts=True,
        num_devices=B,
    )
    x_d = nc.dram_tensor("x", [C, H, W], FP32, kind="ExternalInput").ap()
    p_d = nc.dram_tensor("perm", [2, H, H], FP32, kind="ExternalInput").ap()
    o_d = nc.dram_tensor("out", [C, H, W], FP32, kind="ExternalOutput").ap()
    with TileContext(nc) as tc:
        _emit(nc, tc, x_d, o_d, p_d, f_func=f_func)
    nc.compile()
    _NC_CACHE[key] = nc
    return nc


def _perm_mats():
    pm = np.zeros((2, H, H), dtype=np.float32)
    i = np.arange(H)
    for d, delta in enumerate((1, -1)):
        pm[d, (i // PPC) * PPC + (i % PPC + delta) % PPC, i] = 1.0
    return pm


def run(x, trace=False, tmpdir=None):
    x = np.ascontiguousarray(np.asarray(x), dtype=np.float32)
    assert x.shape == (B, C, H, W), x.shape
    nc = _build()
    pm = _perm_mats()
    in_maps = [{"x": np.ascontiguousarray(x[i]), "perm": pm} for i in range(B)]
    res = run_bass_kernel_spmd(
        nc, in_maps, core_ids=list(range(B)), trace=trace, tmpdir=tmpdir
    )
    out = np.stack([res.results[i]["out"] for i in range(B)], axis=0)
    return out, res


def kernel(x):
    out, _ = run(x)
    return out
